# revision 7
# baseline (speedup 1.0000x reference)
"""Trainium2 Bass kernel for ConditionalEdgeMasker.

Full inputs:
  edge_features      [32, 100000, 16] f32
  ninja_physics_state[32, 18]         f32
  base_edge_mask     [32, 100000]     f32
Output: [32, 100000] f32

Strategy: data-parallel over batch — each of the 8 cores handles 4 batches.
Per-graph physics scalars are folded on the host into a small per-batch
scalar pack (the whole decision tree's scalar-only subexpressions collapse
to 5 numbers). On-chip, the per-edge mask is computed with a fused
elementwise pipeline (tensor_tensor / scalar_tensor_tensor / tensor_scalar)
spread across the Vector, GpSimd and Scalar engines, with boolean planes in
bf16 so the DVE runs its 2x packed mode on them.

Boolean algebra (derived from the reference's elif-chain; t1/t3 are the
argmax==JUMP / ==WALL_SLIDE indicators, A=(can_jump<0.5), d2/d4 the
scalar-only branch decisions, d3e=(vel<min_vel), d5=(ec>2*ke),
over=(maxv>0)&(vel>maxv)):

  sd = over | d3e
     | (t3 & d2)
     | A & (t1 | (c3 & !t3))
     | (!t3 & !c3 & c4 & d4)
     | (t1 & !c3 & !c4 & d5)
  out = base * (1 - sd)

(`under` collapses: over | (~over & d3e) == over | d3e; the d3 branch's
d3e-part is absorbed by the trailing `| d3e`; the `!c1` guards are absorbed
because c1 forces sd=1 anyway.)
"""

import os
import sys

import numpy as np

for _p in ("/opt/trn_rl_repo", os.path.expanduser("~/.axon_site/_ro/trn_rl_repo")):
    if _p not in sys.path and os.path.isdir(_p):
        sys.path.append(_p)

# ---- problem geometry (hardcoded) ----
B, E, F = 32, 100000, 16
NCORES = 8
BPC = B // NCORES  # batches per core
P = 128            # SBUF partitions
EPP = 784          # padded edges per partition; P*EPP = 100352 >= E
EP = P * EPP
NT = 1             # tiles per batch
Q = EPP // NT      # edges per partition per tile
NSC = 8            # scalar-pack slots per batch

_nc = None
LAST_EXEC_NS = None


def _build_nc():
    import concourse.bass as bass
    import concourse.mybir as mybir
    from concourse import tile
    from contextlib import ExitStack

    f32 = mybir.dt.float32
    bf16 = mybir.dt.bfloat16
    AO = mybir.AluOpType

    nc = bass.Bass()
    feats = nc.declare_dram_parameter("feats", [BPC, P, EPP, F], f32, isOutput=False)
    basem = nc.declare_dram_parameter("basem", [BPC, P, EPP], f32, isOutput=False)
    sc32d = nc.declare_dram_parameter("scal32", [P, BPC * NSC], f32, isOutput=False)
    sc16d = nc.declare_dram_parameter("scal16", [P, BPC * NSC], bf16, isOutput=False)
    outm = nc.declare_dram_parameter("outm", [BPC, P, EPP], f32, isOutput=True)

    with tile.TileContext(nc) as tc, ExitStack() as ctx:
        const = ctx.enter_context(tc.tile_pool(name="const", bufs=1))
        raw_p = ctx.enter_context(tc.tile_pool(name="raw", bufs=2))
        io_p = ctx.enter_context(tc.tile_pool(name="io", bufs=2))
        dve_p = ctx.enter_context(tc.tile_pool(name="dve", bufs=1))

        sc32 = const.tile([P, BPC * NSC], f32, name="sc32")
        nc.sync.dma_start(sc32[:, :], sc32d[:, :])
        sc16 = const.tile([P, BPC * NSC], bf16, name="sc16")
        nc.sync.dma_start(sc16[:, :], sc16d[:, :])

        V, G, S = nc.vector, nc.gpsimd, nc.scalar

        for b in range(BPC):
            for t in range(NT):
                sl = slice(t * Q, (t + 1) * Q)
                uid = f"_{b}_{t}"

                R = raw_p.tile([P, Q, F], f32, name="R" + uid, tag="R")
                nc.sync.dma_start(R[:, :, :], feats[b, :, sl, :])
                BM = io_p.tile([P, Q], f32, name="BM" + uid, tag="BM")
                nc.sync.dma_start(BM[:, :], basem[b, :, sl])

                def Fk(k):
                    return R[:, :, k]

                def s32(j):
                    return sc32[:, b * NSC + j : b * NSC + j + 1]

                def s16(j):
                    return sc16[:, b * NSC + j : b * NSC + j + 1]

                def p32(nm, pool=dve_p):
                    return pool.tile([P, Q], f32, name=nm + uid, tag=nm)

                def p16(nm, pool=dve_p):
                    return pool.tile([P, Q], bf16, name=nm + uid, tag=nm)

                # ---- t3 = argmax(f[:6])==3 indicator ----
                m45 = p32("m45")
                V.tensor_tensor(m45, Fk(4), Fk(5), AO.max)
                m01 = p32("m01")
                V.tensor_tensor(m01, Fk(0), Fk(1), AO.max)
                m012 = p32("m012")
                V.tensor_tensor(m012, m01, Fk(2), AO.max)
                a3 = p16("a3")
                V.tensor_tensor(a3, Fk(3), m012, AO.is_gt)
                b3 = p16("b3")
                V.tensor_tensor(b3, Fk(3), m45, AO.is_ge)
                t3 = p16("t3")
                V.tensor_tensor(t3, a3, b3, AO.mult)

                # ---- d3e = (minv > vel) ----
                d3e = p16("d3e")
                V.tensor_scalar(d3e, Fk(12), s32(3), None, AO.is_gt)

                # ---- t1 subtree ----
                m23 = p32("m23")
                V.tensor_tensor(m23, Fk(2), Fk(3), AO.max)
                m2345 = p32("m2345")
                V.tensor_tensor(m2345, m23, m45, AO.max)
                a1 = p16("a1")
                V.tensor_tensor(a1, Fk(1), Fk(0), AO.is_gt)
                b1 = p16("b1")
                V.tensor_tensor(b1, Fk(1), m2345, AO.is_ge)
                t1 = p16("t1")
                V.tensor_tensor(t1, a1, b1, AO.mult)

                # ---- combine ----
                # over-raw: (maxv - vel) * maxv  (negative iff over)
                ovr = p16("ovr")
                V.scalar_tensor_tensor(ovr, Fk(13), s32(3), Fk(13), AO.subtract, AO.mult)
                acc1 = p16("acc1")  # over | d3e
                V.scalar_tensor_tensor(acc1, ovr, 0.0, d3e, AO.is_lt, AO.max)
                acc2 = p16("acc2")  # | t3 & d2
                V.scalar_tensor_tensor(acc2, t3, s16(1), acc1, AO.mult, AO.max)
                c3n = p16("c3n")  # c3 & !t3  ==  (rj>0.5) > t3
                V.scalar_tensor_tensor(c3n, Fk(14), 0.5, t3, AO.is_gt, AO.is_gt)
                u = p16("u")
                V.tensor_tensor(u, t1, c3n, AO.max)
                acc3 = p16("acc3")  # | A & (t1 | c3&!t3)
                V.scalar_tensor_tensor(acc3, u, s16(0), acc2, AO.mult, AO.max)
                nc3 = p16("nc3")
                V.tensor_scalar(nc3, Fk(14), 0.5, None, AO.is_le)
                g = p16("g")  # !c3 & !t3  ==  (rj<=0.5) > t3
                V.scalar_tensor_tensor(g, Fk(14), 0.5, t3, AO.is_le, AO.is_gt)
                c4g = p16("c4g")  # c4 & !c3 & !t3
                V.scalar_tensor_tensor(c4g, Fk(15), 0.5, g, AO.is_gt, AO.mult)
                acc4 = p16("acc4")  # | c4 & !c3 & !t3 & d4
                V.scalar_tensor_tensor(acc4, c4g, s16(2), acc3, AO.mult, AO.max)
                n34 = p16("n34")  # !c4 & !c3
                V.scalar_tensor_tensor(n34, Fk(15), 0.5, nc3, AO.is_le, AO.mult)
                x5a = p16("x5a")  # d5 & !c3 & !c4   (d5 = ec > 2*ke)
                V.scalar_tensor_tensor(x5a, Fk(10), s32(4), n34, AO.is_gt, AO.mult)
                x5b = p16("x5b")
                V.tensor_tensor(x5b, x5a, t1, AO.mult)
                acc5 = p16("acc5")
                V.tensor_tensor(acc5, acc4, x5b, AO.max)

                O = io_p.tile([P, Q], f32, name="O" + uid, tag="O")
                V.scalar_tensor_tensor(O, acc5, 0.5, BM, AO.is_lt, AO.mult)
                nc.sync.dma_start(outm[b, :, sl], O[:, :])

    _split_multi_waits(nc, mybir)
    return nc


def _split_multi_waits(nc, mybir):
    """This walrus build accepts only ONE sync-wait per instruction (any
    format, Drain included). Hoist extra waits into standalone
    EventSemaphore instructions on the same engine, placed just before."""
    n = 0
    for fn in nc.m.functions:
        for bb in fn.blocks:
            insts = list(bb.instructions)
            new_insts = []
            for ins in insts:
                si = ins.sync_info
                if si and si.on_wait and len(si.on_wait) > 1:
                    waits = list(si.on_wait)
                    for w in waits[:-1]:
                        n += 1
                        ev = mybir.InstEventSemaphore(
                            name=f"I-wsplit-{n}", ins=[], outs=[]
                        )
                        ev.engine = ins.engine
                        ev.sync_info = mybir.SyncInfo(on_wait=[w], on_update=[])
                        new_insts.append(ev)
                    ins.sync_info = mybir.SyncInfo(
                        on_wait=[waits[-1]], on_update=list(si.on_update or [])
                    )
                new_insts.append(ins)
            bb.instructions = new_insts


def _get_nc():
    global _nc
    if _nc is None:
        _nc = _build_nc()
    return _nc


def _scalar_pack(ninja_physics_state):
    ph = np.asarray(ninja_physics_state, np.float32)
    vel, wall, ke = ph[:, 2], ph[:, 5], ph[:, 9]
    cj, cwj = ph[:, 16], ph[:, 17]
    A = (cj < 0.5).astype(np.float32)
    d2 = ((wall < 0.5) | (vel < 0.1)).astype(np.float32)
    d4 = ((wall < 0.5) | ((cwj < 0.5) & (vel < 1.0))).astype(np.float32)
    scal = np.zeros((B, NSC), np.float32)
    scal[:, 0] = A
    scal[:, 1] = d2
    scal[:, 2] = d4
    scal[:, 3] = vel
    scal[:, 4] = 2.0 * ke
    scal[:, 5] = -vel
    return scal


def make_in_maps(edge_features, ninja_physics_state, base_edge_mask):
    import ml_dtypes

    feats_pad = np.zeros((B, EP, F), np.float32)
    feats_pad[:, :E, :] = edge_features
    base_pad = np.zeros((B, EP), np.float32)
    base_pad[:, :E] = base_edge_mask
    feats_r = feats_pad.reshape(NCORES, BPC, P, EPP, F)
    base_r = base_pad.reshape(NCORES, BPC, P, EPP)
    scal_c = _scalar_pack(ninja_physics_state).reshape(NCORES, BPC * NSC)

    in_maps = []
    for c in range(NCORES):
        s32t = np.ascontiguousarray(np.broadcast_to(scal_c[c], (P, BPC * NSC)))
        in_maps.append(
            {
                "feats": np.ascontiguousarray(feats_r[c]),
                "basem": np.ascontiguousarray(base_r[c]),
                "scal32": s32t,
                "scal16": s32t.astype(ml_dtypes.bfloat16),
            }
        )
    return in_maps


def kernel(edge_features, ninja_physics_state, base_edge_mask):
    global LAST_EXEC_NS
    from concourse.bass_utils import run_bass_kernel_spmd

    nc = _get_nc()
    in_maps = make_in_maps(edge_features, ninja_physics_state, base_edge_mask)
    res = run_bass_kernel_spmd(nc, in_maps, list(range(NCORES)))
    LAST_EXEC_NS = res.exec_time_ns
    out = np.stack([res.results[c]["outm"] for c in range(NCORES)])
    return np.ascontiguousarray(out.reshape(B, EP)[:, :E].astype(np.float32))
